# revision 21
# baseline (speedup 1.0000x reference)
"""Trainium2 Bass kernel for nn_Baseline_635655160228 (retrieval_knn).

Reference computation (B=64, WAYS=10, SHOTS=5, C=128, H=W=32):
    cov_j = centered-Gram(support_j) / (N-1)          # [ways, C, C], N = shots*hw
    qn    = q / ||q||_2(per channel row)              # [B, C, hw]
    sim[b,j,p] = qn_p^T cov_j qn_p                    # diag quadratic form
    out[b,j]   = sum_p leaky_relu(sim) * conv_w[p]

Key algebraic restructuring:
  cov_j is PSD (Gram of centered data), hence sim >= 0 and LeakyReLU is the
  identity.  Then
      out[b,j] = sum_p w_p qn_p^T cov_j qn_p = <cov_j, W_b>_F
  with W_b = qn diag(w) qn^T a tiny [C,C] matrix per query.
  Mean correction applied at the end:
      out[b,j] = <R_j, W_b> - (1/N) m_j^T W_b m_j     (R raw Gram, m row sums)
  with 1/(N-1) folded into conv_w.

Distribution over 8 NeuronCores — fully collective-free:
  - data-parallel over the query batch (8 queries per core)
  - the support Gram is computed FULLY on every core from a replicated,
    host-prelaid sample-major fp8e4m3 copy of support (6.6 MiB/core).  This
    removes the in-kernel AllReduce entirely: the previous collective-based
    version stalled 40-110us on ncfw staging + cross-core launch skew, which
    dominated the measured span.  fp8 quantization of support adds ~2e-3
    rel err (validated host-side: 3.0e-3 total vs gate 2e-2).
  - the host layout packs a ones-column (c=C) per sample chunk so the Gram
    matmul's rhs yields per-way row sums (for the mean correction) for free,
    and keeps lhsT at exactly 128 columns so FWL (fast weight load) engages.

All bulk matmul operands are fp8/bf16; accumulation stays fp32 in PSUM.
"""

import numpy as np

B, WAYS, SHOTS, C, H, W = 64, 10, 5, 128, 32, 32
HW = H * W                       # 1024
NCORES = 8
BLOC = B // NCORES               # 8 queries per core
NTOT = SHOTS * HW                # 5120 samples per way
NCHUNK = NTOT // 128             # 40 sample chunks of 128 per way
DENOM = float(NTOT - 1)          # 5119
QCH = HW // 128                  # 8 pixel chunks per query

_CACHE = {}


def _build_program():
    import concourse.bass as bass
    import concourse.tile as tile
    from concourse import bacc, mybir

    f32 = mybir.dt.float32
    bf16 = mybir.dt.bfloat16
    fp8 = mybir.dt.float8e4
    AF = mybir.ActivationFunctionType
    ALU = mybir.AluOpType

    nc = bacc.Bacc("TRN2", target_bir_lowering=False, debug=False,
                   num_devices=NCORES)

    q_d = nc.dram_tensor("q", [C, BLOC, HW], bf16, kind="ExternalInput")
    sup_d = nc.dram_tensor("support", [WAYS, 128, NCHUNK * (C + 1)], fp8,
                           kind="ExternalInput")
    w_d = nc.dram_tensor("conv_w", [HW], f32, kind="ExternalInput")
    out_d = nc.dram_tensor("out", [WAYS, BLOC], f32, kind="ExternalOutput")

    with tile.TileContext(nc) as tc:
        with (
            tc.tile_pool(name="const", bufs=1) as constp,
            tc.tile_pool(name="big", bufs=1) as big,
            tc.tile_pool(name="scratch", bufs=2) as scratch,
            tc.tile_pool(name="tp_ps", bufs=3, space="PSUM") as tp_ps,
            tc.tile_pool(name="gram_ps", bufs=2, space="PSUM") as gram_ps,
            tc.tile_pool(name="w_ps", bufs=2, space="PSUM") as w_ps,
            tc.tile_pool(name="fr_ps", bufs=1, space="PSUM") as fr_ps,
        ):
            import ml_dtypes
            ident_d = nc.inline_tensor(
                np.eye(128, dtype=ml_dtypes.bfloat16), name="ident_const")
            ident = constp.tile([128, 128], bf16, tag="ident")

            # block-fold matrix: SEL4[16g + j, g, j] = 1 folds the diagonal
            # [10,8] blocks of the packed Frobenius product
            sel_np = np.zeros((64, 4, WAYS), np.float32)
            for g in range(4):
                for j in range(WAYS):
                    sel_np[16 * g + j, g, j] = 1.0
            sel_d = nc.inline_tensor(sel_np, name="sel_const")
            sel = constp.tile([64, 4, WAYS], f32, tag="sel")

            wp = constp.tile([128, QCH], f32, tag="wp")        # conv_w, p-major
            wps = constp.tile([128, QCH], f32, tag="wps")      # conv_w/(N-1)

            warm_src = constp.tile([128, 256], bf16, tag="warm_src")

            # ---------------- persistent tensors ----------------
            sup_sb = big.tile([128, WAYS, NCHUNK, C + 1], fp8, tag="sup_sb")
            qsb = big.tile([C, BLOC, HW], bf16, tag="qsb")
            qbf = big.tile([C, BLOC, HW], bf16, tag="qbf")
            qT = big.tile([128, BLOC, QCH, C], bf16, tag="qT")
            wqT = big.tile([128, BLOC, QCH, C], bf16, tag="wqT")
            # packed layouts: d = 4p + g so the Frobenius matmul operands
            # [c, (g j)] / [c, (g b)] are contiguous single free dims
            rall_pk = big.tile([C, C // 4, 4, 16], bf16, tag="rall_pk")
            wsb_pk = big.tile([C, C // 4, 4, BLOC], bf16, tag="wsb_pk")
            mcol = constp.tile([C, WAYS], bf16, tag="mcol")

            nsq = constp.tile([128, BLOC], f32, tag="nsq")
            rin = constp.tile([128, BLOC], f32, tag="rin")
            tnw = constp.tile([128, BLOC], f32, tag="tnw")
            mallN = constp.tile([C, WAYS], bf16, tag="mallN")
            msT = constp.tile([WAYS, C], f32, tag="msT")
            ytmp = constp.tile([WAYS, BLOC, C], f32, tag="ytmp")
            ysb = constp.tile([WAYS, BLOC], f32, tag="ysb")
            fin = constp.tile([WAYS, BLOC], f32, tag="fin")

            sup4 = sup_d[:].rearrange("j p (k c) -> j p k c", c=C + 1)

            # ---------------- input DMAs ----------------
            # 3 HW DMA queues sharing a ~300 GB/s pool; full 128-partition
            # APs only.  q ships in quarters interleaved between the early
            # ways so neither the Gram stream nor the norm chain starves;
            # sync's queue (erratic ~8-20us start) gets only mid/late ways.
            nc.vector.memset(warm_src[:], 0.0)
            with tc.high_priority():
                # scalar queue
                nc.scalar.dma_start(sup_sb[:, 0, 0:20, :], sup4[0, :, 0:20, :])
                nc.scalar.dma_start(sup_sb[:, 0, 20:, :], sup4[0, :, 20:, :])
                nc.scalar.dma_start(qsb[:, 0:2, :], q_d[:, 0:2, :])
                nc.scalar.dma_start(sup_sb[:, 1, 0:20, :], sup4[1, :, 0:20, :])
                nc.scalar.dma_start(sup_sb[:, 1, 20:, :], sup4[1, :, 20:, :])
                nc.scalar.dma_start(qsb[:, 2:4, :], q_d[:, 2:4, :])
                nc.scalar.dma_start(sup_sb[:, 3, :, :], sup4[3])
                # gpsimd queue
                nc.gpsimd.dma_start(ident[:], ident_d[:])
                nc.gpsimd.dma_start(sel[:], sel_d[:])
                nc.gpsimd.dma_start(sup_sb[:, 2, 0:20, :], sup4[2, :, 0:20, :])
                nc.gpsimd.dma_start(sup_sb[:, 2, 20:, :], sup4[2, :, 20:, :])
                nc.gpsimd.dma_start(qsb[:, 4:BLOC, :], q_d[:, 4:BLOC, :])
                nc.gpsimd.dma_start(sup_sb[:, 4, :, :], sup4[4])
                nc.gpsimd.dma_start(sup_sb[:, 6, :, :], sup4[6])
                # sync queue
                nc.sync.dma_start(wp[:], w_d.rearrange("(ci p) -> p ci", p=128))
                nc.sync.dma_start(sup_sb[:, 5, :, :], sup4[5])
                nc.sync.dma_start(sup_sb[:, 7, :, :], sup4[7])
                nc.sync.dma_start(sup_sb[:, 8, :, :], sup4[8])
                nc.sync.dma_start(sup_sb[:, 9, :, :], sup4[9])


            nc.vector.tensor_scalar_mul(wps[:], wp[:], 1.0 / DENOM)
            nc.gpsimd.memset(rall_pk[:], 0.0)

            # ---------------- PE warm-up ----------------
            # ~7us of dummy matmuls bridges the gap until the first support
            # chunks land, releasing the HAM clock gate (cold PE = 1.2 GHz).
            warm = fr_ps.tile([128, 256], f32, tag="score")
            last_warm = None
            for wi in range(10):
                last_warm = nc.tensor.matmul(
                    warm[:], lhsT=ident[:], rhs=warm_src[:],
                    start=(wi == 0), stop=(wi == 9))

            # ---------------- stage S: full support Grams (per way) --------
            def gram_copy(j, gp):
                nc.vector.tensor_copy(
                    rall_pk[:, :, :, j],
                    gp[:, 0:C].rearrange("c (p g) -> c p g", g=4))
                nc.vector.tensor_copy(mcol[:, j:j + 1], gp[:, C:C + 1])

            def gram_part(j, gp, k0, k1, first=False):
                # one accumulation group spans both halves of a split way;
                # skip_group_check lets unrelated PE work (transposes, W)
                # interleave between the halves
                for k in range(k0, k1):
                    g_ = nc.tensor.matmul(
                        gp[:], lhsT=sup_sb[:, j, k, 0:C],
                        rhs=sup_sb[:, j, k, :],
                        start=(k == 0), stop=(k == NCHUNK - 1),
                        skip_group_check=(k != 0 and k != NCHUNK - 1))
                    if first and k == 0:
                        tile.add_dep_helper(
                            g_.ins, last_warm.ins,
                            reason="PE warm-up before stage S")

            def gram(j, first=False):
                gp = gram_ps.tile([C, C + 1], f32, tag="gram")
                gram_part(j, gp, 0, NCHUNK, first=first)
                gram_copy(j, gp)

            # ---------------- stage Q pieces ----------------
            def squares(b):
                sq = scratch.tile([C, HW], bf16, tag="sq")
                nc.scalar.activation(sq[:], qsb[:, b, :], AF.Square,
                                     accum_out=nsq[:, b:b + 1])

            def newton(h):
                # rinv = nsq^(-1/2) by Newton from constant seed (nsq ~ 1024)
                s = slice(2 * h, 2 * h + 2)
                r0 = 2.0 ** -5
                nc.vector.tensor_scalar(tnw[:, s], nsq[:, s],
                                        r0 * r0 * -0.5, 1.5,
                                        ALU.mult, ALU.add)
                nc.vector.tensor_scalar_mul(rin[:, s], tnw[:, s], r0)
                for _ in range(2):
                    nc.vector.tensor_mul(tnw[:, s], rin[:, s], rin[:, s])
                    nc.vector.tensor_mul(tnw[:, s], tnw[:, s], nsq[:, s])
                    nc.vector.tensor_scalar(tnw[:, s], tnw[:, s], -0.5, 1.5,
                                            ALU.mult, ALU.add)
                    nc.vector.tensor_mul(rin[:, s], rin[:, s], tnw[:, s])

            def qnorm(b):
                nc.vector.tensor_scalar_mul(qbf[:, b, :], qsb[:, b, :],
                                            rin[:, b:b + 1])

            def tw(b):
                # transpose qn chunks -> qT (ACT group copies from PSUM),
                # then wqT = qT * w' as ONE broadcast multiply on DVE
                # (per-chunk scalar ops cost ~0.4us fixed each — 20us+
                # across the kernel; the broadcast form is one op per query)
                for g in range(2):
                    pt = tp_ps.tile([128, 4, 128], bf16, tag="tp")
                    for i in range(4):
                        ci = 4 * g + i
                        nc.tensor.transpose(
                            pt[:, i, :],
                            qbf[:, b, 128 * ci:128 * (ci + 1)], ident[:])
                    nc.scalar.activation(qT[:, b, 4 * g:4 * g + 4, :], pt[:],
                                         AF.Copy)
                nc.vector.tensor_tensor(
                    wqT[:, b], qT[:, b],
                    wps[:, :, None].to_broadcast((128, QCH, C)),
                    ALU.mult)

            def wmat(b):
                wpt = w_ps.tile([C, C], f32, tag="wacc")
                for ci in range(QCH):
                    nc.tensor.matmul(wpt[:], lhsT=wqT[:, b, ci, :],
                                     rhs=qT[:, b, ci, :],
                                     start=(ci == 0), stop=(ci == QCH - 1))
                nc.vector.tensor_copy(
                    wsb_pk[:, :, :, b],
                    wpt[:].rearrange("c (p g) -> c p g", g=4))

            # PE stream: Grams in natural way order (arrival ~2.3us/way),
            # query norm chain + transposes/W interleaved as inputs land
            gram(0, first=True)

            def qchain(b0):
                squares(b0)
                squares(b0 + 1)
                newton(b0 // 2)
                qnorm(b0)
                qnorm(b0 + 1)

            qchain(0)
            qchain(2)
            qchain(4)
            qchain(6)

            # Grams in arrival order; ways 1/2 are split into k-halves with
            # transposes/W interleaved inside the open accumulation group so
            # no PE wait exceeds the ~3.4us HAM re-throttle window
            tw(0)
            tw(1)
            wmat(0)
            gp1 = gram_ps.tile([C, C + 1], f32, tag="gram")
            gram_part(1, gp1, 0, 20)
            tw(2)
            wmat(1)
            gram_part(1, gp1, 20, NCHUNK)
            gram_copy(1, gp1)
            tw(3)
            wmat(2)
            gp2 = gram_ps.tile([C, C + 1], f32, tag="gram")
            gram_part(2, gp2, 0, 20)
            wmat(3)
            gram_part(2, gp2, 20, NCHUNK)
            gram_copy(2, gp2)
            gram(3)
            tw(4)
            wmat(4)
            tw(5)
            gram(4)
            wmat(5)
            tw(6)
            gram(5)
            wmat(6)
            tw(7)
            gram(6)
            wmat(7)
            gram(7)
            gram(8)
            gram(9)

            # ---------------- Frobenius: score[j,b] = <R_j, W_b> -----------
            # 4 c0-columns packed per matmul (d = 4p+g): lhsT/rhs are the
            # contiguous packed tiles; only the 4 diagonal [10,8] blocks of
            # each [128,32] product are wanted (pads are zeroed), folded by
            # the SEL matmuls below.  32 matmuls instead of a 128-long
            # NX-issue-bound c0 loop.  Runs right after the last Gram; the
            # mean-correction chain overlaps on ACT/DVE.
            score4 = fr_ps.tile([64, 32], f32, tag="score")
            for p in range(C // 4):
                nc.tensor.matmul(
                    score4[:],
                    lhsT=rall_pk[:, p, :, :].rearrange("c g j -> c (g j)"),
                    rhs=wsb_pk[:, p, :, :].rearrange("c g b -> c (g b)"),
                    start=(p == 0), stop=(p == C // 4 - 1))
            scr_sb = constp.tile([64, 32], f32, tag="scr_sb")
            nc.vector.tensor_copy(scr_sb[:], score4[:])

            # ---------------- correction: -(1/N) m^T W_b m ----------------
            # mallN = -m/N  (m = per-way row sums) ; msT = m^T
            nc.scalar.activation(mallN[:], mcol[:], AF.Copy,
                                 scale=-1.0 / NTOT)
            mt = tp_ps.tile([WAYS, C], bf16, tag="tp")
            nc.tensor.transpose(mt[:], mcol[:], ident[:])
            nc.vector.tensor_copy(msT[:], mt[:])
            # u[j,(b,d)] = sum_c (-m[j,c]/N) W[b,c,d] ; y = sum_d u * m[j,d]
            for h in range(2):
                up = w_ps.tile([WAYS, BLOC * C // 2], f32, tag="wacc")
                nc.tensor.matmul(
                    up[:], lhsT=mallN[:],
                    rhs=wsb_pk[:, 16 * h:16 * (h + 1), :, :].rearrange(
                        "c p g b -> c (p g b)"),
                    start=True, stop=True)
                nc.vector.tensor_tensor(
                    ytmp[:, :, 64 * h:64 * (h + 1)].rearrange(
                        "j b (p g) -> j p g b", g=4),
                    up[:].rearrange("j (p g b) -> j p g b", g=4, b=BLOC),
                    msT[:, 64 * h:64 * (h + 1)].rearrange(
                        "j (p g) -> j p g", g=4)[:, :, :, None].to_broadcast(
                        (WAYS, 16, 4, BLOC)),
                    ALU.mult)
            nc.vector.tensor_reduce(ysb[:], ytmp[:],
                                    axis=mybir.AxisListType.X,
                                    op=ALU.add)

            # fold the 4 diagonal blocks on the PE, then add the correction
            fin_ps = w_ps.tile([WAYS, BLOC], f32, tag="wacc")
            for g in range(4):
                nc.tensor.matmul(fin_ps[:], lhsT=sel[:, g, :],
                                 rhs=scr_sb[:, 8 * g:8 * g + 8],
                                 start=(g == 0), stop=(g == 3))
            nc.vector.tensor_add(fin[:], fin_ps[:], ysb[:])
            nc.sync.dma_start(out_d[:], fin[:])

    nc.compile()
    return nc


def _get_program():
    if "nc" not in _CACHE:
        _CACHE["nc"] = _build_program()
    return _CACHE["nc"]


def _make_in_maps(q, support, conv_w):
    import ml_dtypes
    q = np.asarray(q, dtype=np.float32).reshape(B, C, HW)
    qb = q.astype(ml_dtypes.bfloat16)
    # sample-major support: [ways, sample, C] with sample = (shot, pixel),
    # chunked as sample = 128*k + p, laid out [ways, p, k, c] with a ones
    # column at c=C (feeds the row-sum side of the Gram matmul)
    s = np.asarray(support, dtype=np.float32).reshape(WAYS, SHOTS, C, HW)
    s = s.transpose(0, 1, 3, 2).reshape(WAYS, NTOT, C)
    s = s.reshape(WAYS, NCHUNK, 128, C).transpose(0, 2, 1, 3)
    sp = np.empty((WAYS, 128, NCHUNK, C + 1), dtype=ml_dtypes.float8_e4m3)
    sp[..., :C] = s.astype(ml_dtypes.float8_e4m3)
    sp[..., C] = 1.0
    sp = np.ascontiguousarray(sp.reshape(WAYS, 128, NCHUNK * (C + 1)))
    w = np.ascontiguousarray(np.asarray(conv_w, dtype=np.float32))
    in_maps = []
    for k in range(NCORES):
        in_maps.append({
            "q": np.ascontiguousarray(
                qb[k * BLOC:(k + 1) * BLOC].transpose(1, 0, 2)),
            "support": sp,
            "conv_w": w,
        })
    return in_maps


def _run(in_maps, trace=False):
    from concourse.bass_utils import run_bass_kernel_spmd
    nc = _get_program()
    return run_bass_kernel_spmd(nc, in_maps, list(range(NCORES)), trace=trace)


def kernel(q, support, conv_w):
    res = _run(_make_in_maps(q, support, conv_w))
    out = np.concatenate(
        [res.results[k]["out"].T for k in range(NCORES)], axis=0)
    return np.ascontiguousarray(out.astype(np.float32))


# revision 22
# speedup vs baseline: 1.0338x; 1.0338x over previous
"""Trainium2 Bass kernel for nn_Baseline_635655160228 (retrieval_knn).

Reference computation (B=64, WAYS=10, SHOTS=5, C=128, H=W=32):
    cov_j = centered-Gram(support_j) / (N-1)          # [ways, C, C], N = shots*hw
    qn    = q / ||q||_2(per channel row)              # [B, C, hw]
    sim[b,j,p] = qn_p^T cov_j qn_p                    # diag quadratic form
    out[b,j]   = sum_p leaky_relu(sim) * conv_w[p]

Key algebraic restructuring:
  cov_j is PSD (Gram of centered data), hence sim >= 0 and LeakyReLU is the
  identity.  Then
      out[b,j] = sum_p w_p qn_p^T cov_j qn_p = <cov_j, W_b>_F
  with W_b = qn diag(w) qn^T a tiny [C,C] matrix per query.
  Mean correction applied at the end:
      out[b,j] = <R_j, W_b> - (1/N) m_j^T W_b m_j     (R raw Gram, m row sums)
  with 1/(N-1) folded into conv_w.

Distribution over 8 NeuronCores — fully collective-free:
  - data-parallel over the query batch (8 queries per core)
  - the support Gram is computed FULLY on every core from a replicated,
    host-prelaid sample-major fp8e4m3 copy of support (6.6 MiB/core).  This
    removes the in-kernel AllReduce entirely: the previous collective-based
    version stalled 40-110us on ncfw staging + cross-core launch skew, which
    dominated the measured span.  fp8 quantization of support adds ~2e-3
    rel err (validated host-side: 3.0e-3 total vs gate 2e-2).
  - the host layout packs a ones-column (c=C) per sample chunk so the Gram
    matmul's rhs yields per-way row sums (for the mean correction) for free,
    and keeps lhsT at exactly 128 columns so FWL (fast weight load) engages.

All bulk matmul operands are fp8/bf16; accumulation stays fp32 in PSUM.
"""

import numpy as np

B, WAYS, SHOTS, C, H, W = 64, 10, 5, 128, 32, 32
HW = H * W                       # 1024
NCORES = 8
BLOC = B // NCORES               # 8 queries per core
NTOT = SHOTS * HW                # 5120 samples per way
NCHUNK = NTOT // 128             # 40 sample chunks of 128 per way
DENOM = float(NTOT - 1)          # 5119
QCH = HW // 128                  # 8 pixel chunks per query

_CACHE = {}


def _build_program():
    import concourse.bass as bass
    import concourse.tile as tile
    from concourse import bacc, mybir

    f32 = mybir.dt.float32
    bf16 = mybir.dt.bfloat16
    fp8 = mybir.dt.float8e4
    AF = mybir.ActivationFunctionType
    ALU = mybir.AluOpType

    nc = bacc.Bacc("TRN2", target_bir_lowering=False, debug=False,
                   num_devices=NCORES)

    q_d = nc.dram_tensor("q", [C, BLOC, HW], bf16, kind="ExternalInput")
    sup_d = nc.dram_tensor("support", [WAYS, 128, NCHUNK * (C + 1)], fp8,
                           kind="ExternalInput")
    w_d = nc.dram_tensor("conv_w", [HW], f32, kind="ExternalInput")
    out_d = nc.dram_tensor("out", [WAYS, BLOC], f32, kind="ExternalOutput")

    with tile.TileContext(nc) as tc:
        with (
            tc.tile_pool(name="const", bufs=1) as constp,
            tc.tile_pool(name="big", bufs=1) as big,
            tc.tile_pool(name="scratch", bufs=2) as scratch,
            tc.tile_pool(name="tp_ps", bufs=3, space="PSUM") as tp_ps,
            tc.tile_pool(name="gram_ps", bufs=2, space="PSUM") as gram_ps,
            tc.tile_pool(name="w_ps", bufs=2, space="PSUM") as w_ps,
            tc.tile_pool(name="fr_ps", bufs=1, space="PSUM") as fr_ps,
        ):
            import ml_dtypes
            ident_d = nc.inline_tensor(
                np.eye(128, dtype=ml_dtypes.bfloat16), name="ident_const")
            ident = constp.tile([128, 128], bf16, tag="ident")

            # block-fold matrix: SEL4[16g + j, g, j] = 1 folds the diagonal
            # [10,8] blocks of the packed Frobenius product
            sel_np = np.zeros((64, 4, WAYS), np.float32)
            for g in range(4):
                for j in range(WAYS):
                    sel_np[16 * g + j, g, j] = 1.0
            sel_d = nc.inline_tensor(sel_np, name="sel_const")
            sel = constp.tile([64, 4, WAYS], f32, tag="sel")

            wp = constp.tile([128, QCH], f32, tag="wp")        # conv_w, p-major
            wps = constp.tile([128, QCH], f32, tag="wps")      # conv_w/(N-1)

            warm_src = constp.tile([128, 256], bf16, tag="warm_src")

            # ---------------- persistent tensors ----------------
            sup_sb = big.tile([128, WAYS, NCHUNK, C + 1], fp8, tag="sup_sb")
            qsb = big.tile([C, BLOC, HW], bf16, tag="qsb")
            qbf = big.tile([C, BLOC, HW], bf16, tag="qbf")
            qT = big.tile([128, BLOC, QCH, C], bf16, tag="qT")
            wqT = big.tile([128, BLOC, QCH, C], bf16, tag="wqT")
            # packed layouts: d = 4p + g so the Frobenius matmul operands
            # [c, (g j)] / [c, (g b)] are contiguous single free dims
            rall_pk = big.tile([C, C // 4, 4, 16], bf16, tag="rall_pk")
            wsb_pk = big.tile([C, C // 4, 4, BLOC], bf16, tag="wsb_pk")
            mcol = constp.tile([C, WAYS], bf16, tag="mcol")

            nsq = constp.tile([128, BLOC], f32, tag="nsq")
            rin = constp.tile([128, BLOC], f32, tag="rin")
            tnw = constp.tile([128, BLOC], f32, tag="tnw")
            mallN = constp.tile([C, WAYS], bf16, tag="mallN")
            msT = constp.tile([WAYS, C], f32, tag="msT")
            ytmp = constp.tile([WAYS, BLOC, C], f32, tag="ytmp")
            ysb = constp.tile([WAYS, BLOC], f32, tag="ysb")
            fin = constp.tile([WAYS, BLOC], f32, tag="fin")

            sup4 = sup_d[:].rearrange("j p (k c) -> j p k c", c=C + 1)

            # ---------------- input DMAs ----------------
            # 3 HW DMA queues sharing a ~300 GB/s pool; full 128-partition
            # APs only.  q ships in quarters interleaved between the early
            # ways so neither the Gram stream nor the norm chain starves;
            # sync's queue (erratic ~8-20us start) gets only mid/late ways.
            nc.vector.memset(warm_src[:], 0.0)
            with tc.high_priority():
                # scalar queue
                nc.scalar.dma_start(sup_sb[:, 0, 0:20, :], sup4[0, :, 0:20, :])
                nc.scalar.dma_start(sup_sb[:, 0, 20:, :], sup4[0, :, 20:, :])
                nc.scalar.dma_start(qsb[:, 0:2, :], q_d[:, 0:2, :])
                nc.scalar.dma_start(sup_sb[:, 1, 0:20, :], sup4[1, :, 0:20, :])
                nc.scalar.dma_start(sup_sb[:, 1, 20:, :], sup4[1, :, 20:, :])
                nc.scalar.dma_start(qsb[:, 2:4, :], q_d[:, 2:4, :])
                nc.scalar.dma_start(sup_sb[:, 3, :, :], sup4[3])
                # gpsimd queue
                nc.gpsimd.dma_start(ident[:], ident_d[:])
                nc.gpsimd.dma_start(sel[:], sel_d[:])
                nc.gpsimd.dma_start(sup_sb[:, 2, 0:20, :], sup4[2, :, 0:20, :])
                nc.gpsimd.dma_start(sup_sb[:, 2, 20:, :], sup4[2, :, 20:, :])
                nc.gpsimd.dma_start(qsb[:, 4:BLOC, :], q_d[:, 4:BLOC, :])
                nc.gpsimd.dma_start(sup_sb[:, 4, :, :], sup4[4])
                nc.gpsimd.dma_start(sup_sb[:, 6, :, :], sup4[6])
                # sync queue
                nc.sync.dma_start(wp[:], w_d.rearrange("(ci p) -> p ci", p=128))
                nc.sync.dma_start(sup_sb[:, 5, :, :], sup4[5])
                nc.sync.dma_start(sup_sb[:, 7, :, :], sup4[7])
                nc.sync.dma_start(sup_sb[:, 8, :, :], sup4[8])
                nc.sync.dma_start(sup_sb[:, 9, :, :], sup4[9])


            nc.vector.tensor_scalar_mul(wps[:], wp[:], 1.0 / DENOM)
            nc.gpsimd.memset(rall_pk[:], 0.0)

            # ---------------- PE warm-up ----------------
            # ~7us of dummy matmuls bridges the gap until the first support
            # chunks land, releasing the HAM clock gate (cold PE = 1.2 GHz).
            warm = fr_ps.tile([128, 256], f32, tag="score")
            last_warm = None
            for wi in range(10):
                last_warm = nc.tensor.matmul(
                    warm[:], lhsT=ident[:], rhs=warm_src[:],
                    start=(wi == 0), stop=(wi == 9))

            # ---------------- stage S: full support Grams (per way) --------
            def gram_copy(j, gp):
                nc.vector.tensor_copy(
                    rall_pk[:, :, :, j],
                    gp[:, 0:C].rearrange("c (p g) -> c p g", g=4))
                nc.vector.tensor_copy(mcol[:, j:j + 1], gp[:, C:C + 1])

            def gram_part(j, gp, k0, k1, first=False):
                # one accumulation group spans both halves of a split way;
                # skip_group_check lets unrelated PE work (transposes, W)
                # interleave between the halves
                for k in range(k0, k1):
                    g_ = nc.tensor.matmul(
                        gp[:], lhsT=sup_sb[:, j, k, 0:C],
                        rhs=sup_sb[:, j, k, :],
                        start=(k == 0), stop=(k == NCHUNK - 1),
                        skip_group_check=(k != 0 and k != NCHUNK - 1))
                    if first and k == 0:
                        tile.add_dep_helper(
                            g_.ins, last_warm.ins,
                            reason="PE warm-up before stage S")

            def gram(j, first=False):
                gp = gram_ps.tile([C, C + 1], f32, tag="gram")
                gram_part(j, gp, 0, NCHUNK, first=first)
                gram_copy(j, gp)

            # ---------------- stage Q pieces ----------------
            def squares(b):
                sq = scratch.tile([C, HW], bf16, tag="sq")
                nc.scalar.activation(sq[:], qsb[:, b, :], AF.Square,
                                     accum_out=nsq[:, b:b + 1])

            def newton(h):
                # rinv = nsq^(-1/2) by Newton from constant seed (nsq ~ 1024)
                s = slice(2 * h, 2 * h + 2)
                r0 = 2.0 ** -5
                nc.vector.tensor_scalar(tnw[:, s], nsq[:, s],
                                        r0 * r0 * -0.5, 1.5,
                                        ALU.mult, ALU.add)
                nc.vector.tensor_scalar_mul(rin[:, s], tnw[:, s], r0)
                for _ in range(2):
                    nc.vector.tensor_mul(tnw[:, s], rin[:, s], rin[:, s])
                    nc.vector.tensor_mul(tnw[:, s], tnw[:, s], nsq[:, s])
                    nc.vector.tensor_scalar(tnw[:, s], tnw[:, s], -0.5, 1.5,
                                            ALU.mult, ALU.add)
                    nc.vector.tensor_mul(rin[:, s], rin[:, s], tnw[:, s])

            def qnorm(b):
                nc.vector.tensor_scalar_mul(qbf[:, b, :], qsb[:, b, :],
                                            rin[:, b:b + 1])

            def tw(b):
                # transpose qn chunks -> qT (ACT group copies from PSUM),
                # then wqT = qT * w' as ONE broadcast multiply on DVE
                # (per-chunk scalar ops cost ~0.4us fixed each — 20us+
                # across the kernel; the broadcast form is one op per query)
                for g in range(2):
                    pt = tp_ps.tile([128, 4, 128], bf16, tag="tp")
                    for i in range(4):
                        ci = 4 * g + i
                        nc.tensor.transpose(
                            pt[:, i, :],
                            qbf[:, b, 128 * ci:128 * (ci + 1)], ident[:])
                    nc.scalar.activation(qT[:, b, 4 * g:4 * g + 4, :], pt[:],
                                         AF.Copy)
                nc.vector.tensor_tensor(
                    wqT[:, b], qT[:, b],
                    wps[:, :, None].to_broadcast((128, QCH, C)),
                    ALU.mult)

            def wmat(b):
                wpt = w_ps.tile([C, C], f32, tag="wacc")
                for ci in range(QCH):
                    nc.tensor.matmul(wpt[:], lhsT=wqT[:, b, ci, :],
                                     rhs=qT[:, b, ci, :],
                                     start=(ci == 0), stop=(ci == QCH - 1))
                nc.vector.tensor_copy(
                    wsb_pk[:, :, :, b],
                    wpt[:].rearrange("c (p g) -> c p g", g=4))

            # PE stream: Grams in natural way order (arrival ~2.3us/way),
            # query norm chain + transposes/W interleaved as inputs land
            gram(0, first=True)

            def qchain(b0):
                squares(b0)
                squares(b0 + 1)
                newton(b0 // 2)
                qnorm(b0)
                qnorm(b0 + 1)

            qchain(0)
            qchain(2)
            qchain(4)
            qchain(6)

            # Grams in arrival order; tw(b) transposes decoupled from
            # wmat(b) by at least one Gram so the ACT qT copy + DVE wqT
            # multiply complete off the PE critical path
            gram(2)
            gram(1)
            tw(0)
            tw(1)
            wmat(0)
            gram(4)
            wmat(1)
            tw(2)
            tw(3)
            wmat(2)
            gram(3)
            wmat(3)
            tw(4)
            gram(6)
            wmat(4)
            tw(5)
            gram(5)
            wmat(5)
            tw(6)
            gram(7)
            wmat(6)
            tw(7)
            wmat(7)
            gram(8)
            gram(9)

            # ---------------- Frobenius: score[j,b] = <R_j, W_b> -----------
            # 4 c0-columns packed per matmul (d = 4p+g): lhsT/rhs are the
            # contiguous packed tiles; only the 4 diagonal [10,8] blocks of
            # each [128,32] product are wanted (pads are zeroed), folded by
            # the SEL matmuls below.  32 matmuls instead of a 128-long
            # NX-issue-bound c0 loop.  Runs right after the last Gram; the
            # mean-correction chain overlaps on ACT/DVE.
            score4 = fr_ps.tile([64, 32], f32, tag="score")
            for p in range(C // 4):
                nc.tensor.matmul(
                    score4[:],
                    lhsT=rall_pk[:, p, :, :].rearrange("c g j -> c (g j)"),
                    rhs=wsb_pk[:, p, :, :].rearrange("c g b -> c (g b)"),
                    start=(p == 0), stop=(p == C // 4 - 1))
            scr_sb = constp.tile([64, 32], f32, tag="scr_sb")
            nc.vector.tensor_copy(scr_sb[:], score4[:])

            # ---------------- correction: -(1/N) m^T W_b m ----------------
            # mallN = -m/N  (m = per-way row sums) ; msT = m^T
            nc.scalar.activation(mallN[:], mcol[:], AF.Copy,
                                 scale=-1.0 / NTOT)
            mt = tp_ps.tile([WAYS, C], bf16, tag="tp")
            nc.tensor.transpose(mt[:], mcol[:], ident[:])
            nc.vector.tensor_copy(msT[:], mt[:])
            # u[j,(b,d)] = sum_c (-m[j,c]/N) W[b,c,d] ; y = sum_d u * m[j,d]
            for h in range(2):
                up = w_ps.tile([WAYS, BLOC * C // 2], f32, tag="wacc")
                nc.tensor.matmul(
                    up[:], lhsT=mallN[:],
                    rhs=wsb_pk[:, 16 * h:16 * (h + 1), :, :].rearrange(
                        "c p g b -> c (p g b)"),
                    start=True, stop=True)
                nc.vector.tensor_tensor(
                    ytmp[:, :, 64 * h:64 * (h + 1)].rearrange(
                        "j b (p g) -> j p g b", g=4),
                    up[:].rearrange("j (p g b) -> j p g b", g=4, b=BLOC),
                    msT[:, 64 * h:64 * (h + 1)].rearrange(
                        "j (p g) -> j p g", g=4)[:, :, :, None].to_broadcast(
                        (WAYS, 16, 4, BLOC)),
                    ALU.mult)
            nc.vector.tensor_reduce(ysb[:], ytmp[:],
                                    axis=mybir.AxisListType.X,
                                    op=ALU.add)

            # fold the 4 diagonal blocks on the PE, then add the correction
            fin_ps = w_ps.tile([WAYS, BLOC], f32, tag="wacc")
            for g in range(4):
                nc.tensor.matmul(fin_ps[:], lhsT=sel[:, g, :],
                                 rhs=scr_sb[:, 8 * g:8 * g + 8],
                                 start=(g == 0), stop=(g == 3))
            nc.vector.tensor_add(fin[:], fin_ps[:], ysb[:])
            nc.sync.dma_start(out_d[:], fin[:])

    nc.compile()
    return nc


def _get_program():
    if "nc" not in _CACHE:
        _CACHE["nc"] = _build_program()
    return _CACHE["nc"]


def _make_in_maps(q, support, conv_w):
    import ml_dtypes
    q = np.asarray(q, dtype=np.float32).reshape(B, C, HW)
    qb = q.astype(ml_dtypes.bfloat16)
    # sample-major support: [ways, sample, C] with sample = (shot, pixel),
    # chunked as sample = 128*k + p, laid out [ways, p, k, c] with a ones
    # column at c=C (feeds the row-sum side of the Gram matmul)
    s = np.asarray(support, dtype=np.float32).reshape(WAYS, SHOTS, C, HW)
    s = s.transpose(0, 1, 3, 2).reshape(WAYS, NTOT, C)
    s = s.reshape(WAYS, NCHUNK, 128, C).transpose(0, 2, 1, 3)
    sp = np.empty((WAYS, 128, NCHUNK, C + 1), dtype=ml_dtypes.float8_e4m3)
    sp[..., :C] = s.astype(ml_dtypes.float8_e4m3)
    sp[..., C] = 1.0
    sp = np.ascontiguousarray(sp.reshape(WAYS, 128, NCHUNK * (C + 1)))
    w = np.ascontiguousarray(np.asarray(conv_w, dtype=np.float32))
    in_maps = []
    for k in range(NCORES):
        in_maps.append({
            "q": np.ascontiguousarray(
                qb[k * BLOC:(k + 1) * BLOC].transpose(1, 0, 2)),
            "support": sp,
            "conv_w": w,
        })
    return in_maps


def _run(in_maps, trace=False):
    from concourse.bass_utils import run_bass_kernel_spmd
    nc = _get_program()
    return run_bass_kernel_spmd(nc, in_maps, list(range(NCORES)), trace=trace)


def kernel(q, support, conv_w):
    res = _run(_make_in_maps(q, support, conv_w))
    out = np.concatenate(
        [res.results[k]["out"].T for k in range(NCORES)], axis=0)
    return np.ascontiguousarray(out.astype(np.float32))


# revision 23
# speedup vs baseline: 1.0666x; 1.0317x over previous
"""Trainium2 Bass kernel for nn_Baseline_635655160228 (retrieval_knn).

Reference computation (B=64, WAYS=10, SHOTS=5, C=128, H=W=32):
    cov_j = centered-Gram(support_j) / (N-1)          # [ways, C, C], N = shots*hw
    qn    = q / ||q||_2(per channel row)              # [B, C, hw]
    sim[b,j,p] = qn_p^T cov_j qn_p                    # diag quadratic form
    out[b,j]   = sum_p leaky_relu(sim) * conv_w[p]

Key algebraic restructuring:
  cov_j is PSD (Gram of centered data), hence sim >= 0 and LeakyReLU is the
  identity.  Then
      out[b,j] = sum_p w_p qn_p^T cov_j qn_p = <cov_j, W_b>_F
  with W_b = qn diag(w) qn^T a tiny [C,C] matrix per query.
  Mean correction applied at the end:
      out[b,j] = <R_j, W_b> - (1/N) m_j^T W_b m_j     (R raw Gram, m row sums)
  with 1/(N-1) folded into conv_w.

Distribution over 8 NeuronCores — fully collective-free:
  - data-parallel over the query batch (8 queries per core)
  - the support Gram is computed FULLY on every core from a replicated,
    host-prelaid sample-major fp8e4m3 copy of support (6.6 MiB/core).  This
    removes the in-kernel AllReduce entirely: the previous collective-based
    version stalled 40-110us on ncfw staging + cross-core launch skew, which
    dominated the measured span.  fp8 quantization of support adds ~2e-3
    rel err (validated host-side: 3.0e-3 total vs gate 2e-2).
  - the host layout packs a ones-column (c=C) per sample chunk so the Gram
    matmul's rhs yields per-way row sums (for the mean correction) for free,
    and keeps lhsT at exactly 128 columns so FWL (fast weight load) engages.

All bulk matmul operands are fp8/bf16; accumulation stays fp32 in PSUM.
"""

import numpy as np

B, WAYS, SHOTS, C, H, W = 64, 10, 5, 128, 32, 32
HW = H * W                       # 1024
NCORES = 8
BLOC = B // NCORES               # 8 queries per core
NTOT = SHOTS * HW                # 5120 samples per way
NCHUNK = NTOT // 128             # 40 sample chunks of 128 per way
DENOM = float(NTOT - 1)          # 5119
QCH = HW // 128                  # 8 pixel chunks per query

_CACHE = {}


def _build_program():
    import concourse.bass as bass
    import concourse.tile as tile
    from concourse import bacc, mybir

    f32 = mybir.dt.float32
    bf16 = mybir.dt.bfloat16
    fp8 = mybir.dt.float8e4
    AF = mybir.ActivationFunctionType
    ALU = mybir.AluOpType

    nc = bacc.Bacc("TRN2", target_bir_lowering=False, debug=False,
                   num_devices=1)

    q_d = nc.dram_tensor("q", [C, BLOC, HW], bf16, kind="ExternalInput")
    sup_d = nc.dram_tensor("support", [WAYS, 128, NCHUNK * (C + 1)], fp8,
                           kind="ExternalInput")
    w_d = nc.dram_tensor("conv_w", [HW], f32, kind="ExternalInput")
    out_d = nc.dram_tensor("out", [WAYS, BLOC], f32, kind="ExternalOutput")

    with tile.TileContext(nc) as tc:
        with (
            tc.tile_pool(name="const", bufs=1) as constp,
            tc.tile_pool(name="big", bufs=1) as big,
            tc.tile_pool(name="scratch", bufs=2) as scratch,
            tc.tile_pool(name="tp_ps", bufs=3, space="PSUM") as tp_ps,
            tc.tile_pool(name="gram_ps", bufs=2, space="PSUM") as gram_ps,
            tc.tile_pool(name="w_ps", bufs=2, space="PSUM") as w_ps,
            tc.tile_pool(name="fr_ps", bufs=1, space="PSUM") as fr_ps,
        ):
            import ml_dtypes
            ident_d = nc.inline_tensor(
                np.eye(128, dtype=ml_dtypes.bfloat16), name="ident_const")
            ident = constp.tile([128, 128], bf16, tag="ident")

            # block-fold matrix: SEL4[16g + j, g, j] = 1 folds the diagonal
            # [10,8] blocks of the packed Frobenius product
            sel_np = np.zeros((64, 4, WAYS), np.float32)
            for g in range(4):
                for j in range(WAYS):
                    sel_np[16 * g + j, g, j] = 1.0
            sel_d = nc.inline_tensor(sel_np, name="sel_const")
            sel = constp.tile([64, 4, WAYS], f32, tag="sel")

            wp = constp.tile([128, QCH], f32, tag="wp")        # conv_w, p-major
            wps = constp.tile([128, QCH], f32, tag="wps")      # conv_w/(N-1)

            warm_src = constp.tile([128, 256], bf16, tag="warm_src")

            # ---------------- persistent tensors ----------------
            sup_sb = big.tile([128, WAYS, NCHUNK, C + 1], fp8, tag="sup_sb")
            qsb = big.tile([C, BLOC, HW], bf16, tag="qsb")
            qbf = big.tile([C, BLOC, HW], bf16, tag="qbf")
            qT = big.tile([128, BLOC, QCH, C], bf16, tag="qT")
            wqT = big.tile([128, BLOC, QCH, C], bf16, tag="wqT")
            # packed layouts: d = 4p + g so the Frobenius matmul operands
            # [c, (g j)] / [c, (g b)] are contiguous single free dims
            rall_pk = big.tile([C, C // 4, 4, 16], bf16, tag="rall_pk")
            wsb_pk = big.tile([C, C // 4, 4, BLOC], bf16, tag="wsb_pk")
            mcol = constp.tile([C, WAYS], bf16, tag="mcol")

            nsq = constp.tile([128, BLOC], f32, tag="nsq")
            rin = constp.tile([128, BLOC], f32, tag="rin")
            tnw = constp.tile([128, BLOC], f32, tag="tnw")
            mallN = constp.tile([C, WAYS], bf16, tag="mallN")
            msT = constp.tile([WAYS, C], f32, tag="msT")
            ytmp = constp.tile([WAYS, BLOC, C], f32, tag="ytmp")
            ysb = constp.tile([WAYS, BLOC], f32, tag="ysb")
            fin = constp.tile([WAYS, BLOC], f32, tag="fin")

            sup4 = sup_d[:].rearrange("j p (k c) -> j p k c", c=C + 1)

            # ---------------- input DMAs ----------------
            # 3 HW DMA queues sharing a ~300 GB/s pool; full 128-partition
            # APs only.  q ships in quarters interleaved between the early
            # ways so neither the Gram stream nor the norm chain starves;
            # sync's queue (erratic ~8-20us start) gets only mid/late ways.
            nc.vector.memset(warm_src[:], 0.0)
            with tc.high_priority():
                # scalar queue
                nc.scalar.dma_start(sup_sb[:, 0, 0:20, :], sup4[0, :, 0:20, :])
                nc.scalar.dma_start(sup_sb[:, 0, 20:, :], sup4[0, :, 20:, :])
                nc.scalar.dma_start(qsb[:, 0:2, :], q_d[:, 0:2, :])
                nc.scalar.dma_start(sup_sb[:, 1, 0:20, :], sup4[1, :, 0:20, :])
                nc.scalar.dma_start(sup_sb[:, 1, 20:, :], sup4[1, :, 20:, :])
                nc.scalar.dma_start(qsb[:, 2:4, :], q_d[:, 2:4, :])
                nc.scalar.dma_start(sup_sb[:, 3, :, :], sup4[3])
                # gpsimd queue
                nc.gpsimd.dma_start(ident[:], ident_d[:])
                nc.gpsimd.dma_start(sel[:], sel_d[:])
                nc.gpsimd.dma_start(sup_sb[:, 2, 0:20, :], sup4[2, :, 0:20, :])
                nc.gpsimd.dma_start(sup_sb[:, 2, 20:, :], sup4[2, :, 20:, :])
                nc.gpsimd.dma_start(qsb[:, 4:BLOC, :], q_d[:, 4:BLOC, :])
                nc.gpsimd.dma_start(sup_sb[:, 4, :, :], sup4[4])
                nc.gpsimd.dma_start(sup_sb[:, 6, :, :], sup4[6])
                # sync queue
                nc.sync.dma_start(wp[:], w_d.rearrange("(ci p) -> p ci", p=128))
                nc.sync.dma_start(sup_sb[:, 5, :, :], sup4[5])
                nc.sync.dma_start(sup_sb[:, 7, :, :], sup4[7])
                nc.sync.dma_start(sup_sb[:, 8, :, :], sup4[8])
                nc.sync.dma_start(sup_sb[:, 9, :, :], sup4[9])


            nc.vector.tensor_scalar_mul(wps[:], wp[:], 1.0 / DENOM)
            nc.gpsimd.memset(rall_pk[:], 0.0)

            # ---------------- PE warm-up ----------------
            # ~7us of dummy matmuls bridges the gap until the first support
            # chunks land, releasing the HAM clock gate (cold PE = 1.2 GHz).
            warm = fr_ps.tile([128, 256], f32, tag="score")
            last_warm = None
            for wi in range(10):
                last_warm = nc.tensor.matmul(
                    warm[:], lhsT=ident[:], rhs=warm_src[:],
                    start=(wi == 0), stop=(wi == 9))

            # ---------------- stage S: full support Grams (per way) --------
            def gram_copy(j, gp):
                nc.vector.tensor_copy(
                    rall_pk[:, :, :, j],
                    gp[:, 0:C].rearrange("c (p g) -> c p g", g=4))
                nc.vector.tensor_copy(mcol[:, j:j + 1], gp[:, C:C + 1])

            def gram_part(j, gp, k0, k1, first=False):
                # one accumulation group spans both halves of a split way;
                # skip_group_check lets unrelated PE work (transposes, W)
                # interleave between the halves
                for k in range(k0, k1):
                    g_ = nc.tensor.matmul(
                        gp[:], lhsT=sup_sb[:, j, k, 0:C],
                        rhs=sup_sb[:, j, k, :],
                        start=(k == 0), stop=(k == NCHUNK - 1),
                        skip_group_check=(k != 0 and k != NCHUNK - 1))
                    if first and k == 0:
                        tile.add_dep_helper(
                            g_.ins, last_warm.ins,
                            reason="PE warm-up before stage S")

            def gram(j, first=False):
                gp = gram_ps.tile([C, C + 1], f32, tag="gram")
                gram_part(j, gp, 0, NCHUNK, first=first)
                gram_copy(j, gp)

            # ---------------- stage Q pieces ----------------
            def squares(b):
                sq = scratch.tile([C, HW], bf16, tag="sq")
                nc.scalar.activation(sq[:], qsb[:, b, :], AF.Square,
                                     accum_out=nsq[:, b:b + 1])

            def newton(h):
                # rinv = nsq^(-1/2) by Newton from constant seed (nsq ~ 1024)
                s = slice(2 * h, 2 * h + 2)
                r0 = 2.0 ** -5
                nc.vector.tensor_scalar(tnw[:, s], nsq[:, s],
                                        r0 * r0 * -0.5, 1.5,
                                        ALU.mult, ALU.add)
                nc.vector.tensor_scalar_mul(rin[:, s], tnw[:, s], r0)
                for _ in range(2):
                    nc.vector.tensor_mul(tnw[:, s], rin[:, s], rin[:, s])
                    nc.vector.tensor_mul(tnw[:, s], tnw[:, s], nsq[:, s])
                    nc.vector.tensor_scalar(tnw[:, s], tnw[:, s], -0.5, 1.5,
                                            ALU.mult, ALU.add)
                    nc.vector.tensor_mul(rin[:, s], rin[:, s], tnw[:, s])

            def qnorm(b):
                nc.vector.tensor_scalar_mul(qbf[:, b, :], qsb[:, b, :],
                                            rin[:, b:b + 1])

            def tw(b):
                # transpose qn chunks -> qT (ACT group copies from PSUM),
                # then wqT = qT * w' as ONE broadcast multiply on DVE
                # (per-chunk scalar ops cost ~0.4us fixed each — 20us+
                # across the kernel; the broadcast form is one op per query)
                for g in range(2):
                    pt = tp_ps.tile([128, 4, 128], bf16, tag="tp")
                    for i in range(4):
                        ci = 4 * g + i
                        nc.tensor.transpose(
                            pt[:, i, :],
                            qbf[:, b, 128 * ci:128 * (ci + 1)], ident[:])
                    nc.scalar.activation(qT[:, b, 4 * g:4 * g + 4, :], pt[:],
                                         AF.Copy)
                nc.vector.tensor_tensor(
                    wqT[:, b], qT[:, b],
                    wps[:, :, None].to_broadcast((128, QCH, C)),
                    ALU.mult)

            def wmat(b):
                wpt = w_ps.tile([C, C], f32, tag="wacc")
                for ci in range(QCH):
                    nc.tensor.matmul(wpt[:], lhsT=wqT[:, b, ci, :],
                                     rhs=qT[:, b, ci, :],
                                     start=(ci == 0), stop=(ci == QCH - 1))
                nc.vector.tensor_copy(
                    wsb_pk[:, :, :, b],
                    wpt[:].rearrange("c (p g) -> c p g", g=4))

            # PE stream: Grams in natural way order (arrival ~2.3us/way),
            # query norm chain + transposes/W interleaved as inputs land
            gram(0, first=True)

            def qchain(b0):
                squares(b0)
                squares(b0 + 1)
                newton(b0 // 2)
                qnorm(b0)
                qnorm(b0 + 1)

            qchain(0)
            qchain(2)
            qchain(4)
            qchain(6)

            # Grams in arrival order; tw(b) transposes decoupled from
            # wmat(b) by at least one Gram so the ACT qT copy + DVE wqT
            # multiply complete off the PE critical path
            gram(2)
            gram(1)
            tw(0)
            tw(1)
            wmat(0)
            gram(4)
            wmat(1)
            tw(2)
            tw(3)
            wmat(2)
            gram(3)
            wmat(3)
            tw(4)
            gram(6)
            wmat(4)
            tw(5)
            gram(5)
            wmat(5)
            tw(6)
            gram(7)
            wmat(6)
            tw(7)
            wmat(7)
            gram(8)
            gram(9)

            # ---------------- Frobenius: score[j,b] = <R_j, W_b> -----------
            # 4 c0-columns packed per matmul (d = 4p+g): lhsT/rhs are the
            # contiguous packed tiles; only the 4 diagonal [10,8] blocks of
            # each [128,32] product are wanted (pads are zeroed), folded by
            # the SEL matmuls below.  32 matmuls instead of a 128-long
            # NX-issue-bound c0 loop.  Runs right after the last Gram; the
            # mean-correction chain overlaps on ACT/DVE.
            score4 = fr_ps.tile([64, 32], f32, tag="score")
            for p in range(C // 4):
                nc.tensor.matmul(
                    score4[:],
                    lhsT=rall_pk[:, p, :, :].rearrange("c g j -> c (g j)"),
                    rhs=wsb_pk[:, p, :, :].rearrange("c g b -> c (g b)"),
                    start=(p == 0), stop=(p == C // 4 - 1))
            scr_sb = constp.tile([64, 32], f32, tag="scr_sb")
            nc.vector.tensor_copy(scr_sb[:], score4[:])

            # ---------------- correction: -(1/N) m^T W_b m ----------------
            # mallN = -m/N  (m = per-way row sums) ; msT = m^T
            nc.scalar.activation(mallN[:], mcol[:], AF.Copy,
                                 scale=-1.0 / NTOT)
            mt = tp_ps.tile([WAYS, C], bf16, tag="tp")
            nc.tensor.transpose(mt[:], mcol[:], ident[:])
            nc.vector.tensor_copy(msT[:], mt[:])
            # u[j,(b,d)] = sum_c (-m[j,c]/N) W[b,c,d] ; y = sum_d u * m[j,d]
            for h in range(2):
                up = w_ps.tile([WAYS, BLOC * C // 2], f32, tag="wacc")
                nc.tensor.matmul(
                    up[:], lhsT=mallN[:],
                    rhs=wsb_pk[:, 16 * h:16 * (h + 1), :, :].rearrange(
                        "c p g b -> c (p g b)"),
                    start=True, stop=True)
                nc.vector.tensor_tensor(
                    ytmp[:, :, 64 * h:64 * (h + 1)].rearrange(
                        "j b (p g) -> j p g b", g=4),
                    up[:].rearrange("j (p g b) -> j p g b", g=4, b=BLOC),
                    msT[:, 64 * h:64 * (h + 1)].rearrange(
                        "j (p g) -> j p g", g=4)[:, :, :, None].to_broadcast(
                        (WAYS, 16, 4, BLOC)),
                    ALU.mult)
            nc.vector.tensor_reduce(ysb[:], ytmp[:],
                                    axis=mybir.AxisListType.X,
                                    op=ALU.add)

            # fold the 4 diagonal blocks on the PE, then add the correction
            fin_ps = w_ps.tile([WAYS, BLOC], f32, tag="wacc")
            for g in range(4):
                nc.tensor.matmul(fin_ps[:], lhsT=sel[:, g, :],
                                 rhs=scr_sb[:, 8 * g:8 * g + 8],
                                 start=(g == 0), stop=(g == 3))
            nc.vector.tensor_add(fin[:], fin_ps[:], ysb[:])
            nc.sync.dma_start(out_d[:], fin[:])

    nc.compile()
    return nc


def _get_program():
    if "nc" not in _CACHE:
        _CACHE["nc"] = _build_program()
    return _CACHE["nc"]


def _make_in_maps(q, support, conv_w):
    import ml_dtypes
    q = np.asarray(q, dtype=np.float32).reshape(B, C, HW)
    qb = q.astype(ml_dtypes.bfloat16)
    # sample-major support: [ways, sample, C] with sample = (shot, pixel),
    # chunked as sample = 128*k + p, laid out [ways, p, k, c] with a ones
    # column at c=C (feeds the row-sum side of the Gram matmul)
    s = np.asarray(support, dtype=np.float32).reshape(WAYS, SHOTS, C, HW)
    s = s.transpose(0, 1, 3, 2).reshape(WAYS, NTOT, C)
    s = s.reshape(WAYS, NCHUNK, 128, C).transpose(0, 2, 1, 3)
    sp = np.empty((WAYS, 128, NCHUNK, C + 1), dtype=ml_dtypes.float8_e4m3)
    sp[..., :C] = s.astype(ml_dtypes.float8_e4m3)
    sp[..., C] = 1.0
    sp = np.ascontiguousarray(sp.reshape(WAYS, 128, NCHUNK * (C + 1)))
    w = np.ascontiguousarray(np.asarray(conv_w, dtype=np.float32))
    in_maps = []
    for k in range(NCORES):
        in_maps.append({
            "q": np.ascontiguousarray(
                qb[k * BLOC:(k + 1) * BLOC].transpose(1, 0, 2)),
            "support": sp,
            "conv_w": w,
        })
    return in_maps


def _run(in_maps, trace=False):
    from concourse.bass_utils import run_bass_kernel_spmd
    nc = _get_program()
    return run_bass_kernel_spmd(nc, in_maps, list(range(NCORES)), trace=trace)


def kernel(q, support, conv_w):
    res = _run(_make_in_maps(q, support, conv_w))
    out = np.concatenate(
        [res.results[k]["out"].T for k in range(NCORES)], axis=0)
    return np.ascontiguousarray(out.astype(np.float32))


# revision 24
# speedup vs baseline: 1.0866x; 1.0188x over previous
"""Trainium2 Bass kernel for nn_Baseline_635655160228 (retrieval_knn).

Reference computation (B=64, WAYS=10, SHOTS=5, C=128, H=W=32):
    cov_j = centered-Gram(support_j) / (N-1)          # [ways, C, C], N = shots*hw
    qn    = q / ||q||_2(per channel row)              # [B, C, hw]
    sim[b,j,p] = qn_p^T cov_j qn_p                    # diag quadratic form
    out[b,j]   = sum_p leaky_relu(sim) * conv_w[p]

Key algebraic restructuring:
  cov_j is PSD (Gram of centered data), hence sim >= 0 and LeakyReLU is the
  identity.  Then
      out[b,j] = sum_p w_p qn_p^T cov_j qn_p = <cov_j, W_b>_F
  with W_b = qn diag(w) qn^T a tiny [C,C] matrix per query.
  Mean correction applied at the end:
      out[b,j] = <R_j, W_b> - (1/N) m_j^T W_b m_j     (R raw Gram, m row sums)
  with 1/(N-1) folded into conv_w.

Distribution over 8 NeuronCores — fully collective-free:
  - data-parallel over the query batch (8 queries per core)
  - the support Gram is computed FULLY on every core from a replicated,
    host-prelaid sample-major fp8e4m3 copy of support (6.6 MiB/core).  This
    removes the in-kernel AllReduce entirely: the previous collective-based
    version stalled 40-110us on ncfw staging + cross-core launch skew, which
    dominated the measured span.  fp8 quantization of support adds ~2e-3
    rel err (validated host-side: 3.0e-3 total vs gate 2e-2).
  - the host layout packs a ones-column (c=C) per sample chunk so the Gram
    matmul's rhs yields per-way row sums (for the mean correction) for free,
    and keeps lhsT at exactly 128 columns so FWL (fast weight load) engages.

All bulk matmul operands are fp8/bf16; accumulation stays fp32 in PSUM.
"""

import numpy as np

B, WAYS, SHOTS, C, H, W = 64, 10, 5, 128, 32, 32
HW = H * W                       # 1024
NCORES = 8
BLOC = B // NCORES               # 8 queries per core
NTOT = SHOTS * HW                # 5120 samples per way
NCHUNK = NTOT // 128             # 40 sample chunks of 128 per way
DENOM = float(NTOT - 1)          # 5119
QCH = HW // 128                  # 8 pixel chunks per query

_CACHE = {}


def _build_program():
    import concourse.bass as bass
    import concourse.tile as tile
    from concourse import bacc, mybir

    f32 = mybir.dt.float32
    bf16 = mybir.dt.bfloat16
    fp8 = mybir.dt.float8e4
    AF = mybir.ActivationFunctionType
    ALU = mybir.AluOpType

    nc = bacc.Bacc("TRN2", target_bir_lowering=False, debug=False,
                   num_devices=1)

    q_d = nc.dram_tensor("q", [C, BLOC, HW], bf16, kind="ExternalInput")
    sup_d = nc.dram_tensor("support", [WAYS, 128, NCHUNK * (C + 1)], fp8,
                           kind="ExternalInput")
    w_d = nc.dram_tensor("conv_w", [HW], f32, kind="ExternalInput")
    out_d = nc.dram_tensor("out", [WAYS, BLOC], f32, kind="ExternalOutput")

    with tile.TileContext(nc) as tc:
        with (
            tc.tile_pool(name="const", bufs=1) as constp,
            tc.tile_pool(name="big", bufs=1) as big,
            tc.tile_pool(name="scratch", bufs=2) as scratch,
            tc.tile_pool(name="tp_ps", bufs=3, space="PSUM") as tp_ps,
            tc.tile_pool(name="gram_ps", bufs=2, space="PSUM") as gram_ps,
            tc.tile_pool(name="w_ps", bufs=2, space="PSUM") as w_ps,
            tc.tile_pool(name="fr_ps", bufs=1, space="PSUM") as fr_ps,
        ):
            import ml_dtypes
            ident_d = nc.inline_tensor(
                np.eye(128, dtype=ml_dtypes.bfloat16), name="ident_const")
            ident = constp.tile([128, 128], bf16, tag="ident")

            # block-fold matrix: SEL4[16g + j, g, j] = 1 folds the diagonal
            # [10,8] blocks of the packed Frobenius product
            sel_np = np.zeros((64, 4, WAYS), np.float32)
            for g in range(4):
                for j in range(WAYS):
                    sel_np[16 * g + j, g, j] = 1.0
            sel_d = nc.inline_tensor(sel_np, name="sel_const")
            sel = constp.tile([64, 4, WAYS], f32, tag="sel")

            wp = constp.tile([128, QCH], f32, tag="wp")        # conv_w, p-major
            wps = constp.tile([128, QCH], f32, tag="wps")      # conv_w/(N-1)

            warm_src = constp.tile([128, 256], bf16, tag="warm_src")

            # ---------------- persistent tensors ----------------
            sup_sb = big.tile([128, WAYS, NCHUNK, C + 1], fp8, tag="sup_sb")
            qsb = big.tile([C, BLOC, HW], bf16, tag="qsb")
            qbf = big.tile([C, BLOC, HW], bf16, tag="qbf")
            qT = big.tile([128, BLOC, QCH, C], bf16, tag="qT")
            wqT = big.tile([128, BLOC, QCH, C], bf16, tag="wqT")
            # packed layouts: d = 4p + g so the Frobenius matmul operands
            # [c, (g j)] / [c, (g b)] are contiguous single free dims
            rall_pk = big.tile([C, C // 4, 4, 16], bf16, tag="rall_pk")
            wsb_pk = big.tile([C, C // 4, 4, BLOC], bf16, tag="wsb_pk")
            mcol = constp.tile([C, WAYS], bf16, tag="mcol")

            nsq = constp.tile([128, BLOC], f32, tag="nsq")
            rin = constp.tile([128, BLOC], f32, tag="rin")
            tnw = constp.tile([128, BLOC], f32, tag="tnw")
            mallN = constp.tile([C, WAYS], bf16, tag="mallN")
            msT = constp.tile([WAYS, C], f32, tag="msT")
            ytmp = constp.tile([WAYS, BLOC, C], f32, tag="ytmp")
            ysb = constp.tile([WAYS, BLOC], f32, tag="ysb")
            fin = constp.tile([WAYS, BLOC], f32, tag="fin")

            sup4 = sup_d[:].rearrange("j p (k c) -> j p k c", c=C + 1)

            # ---------------- input DMAs ----------------
            # 3 HW DMA queues sharing a ~300 GB/s pool; full 128-partition
            # APs only.  q ships in quarters interleaved between the early
            # ways so neither the Gram stream nor the norm chain starves;
            # sync's queue (erratic ~8-20us start) gets only mid/late ways.
            nc.vector.memset(warm_src[:], 0.0)
            with tc.high_priority():
                # scalar queue
                nc.scalar.dma_start(sup_sb[:, 0, 0:20, :], sup4[0, :, 0:20, :])
                nc.scalar.dma_start(sup_sb[:, 0, 20:, :], sup4[0, :, 20:, :])
                nc.scalar.dma_start(qsb[:, 0:2, :], q_d[:, 0:2, :])
                nc.scalar.dma_start(qsb[:, 2:4, :], q_d[:, 2:4, :])
                nc.scalar.dma_start(sup_sb[:, 3, :, :], sup4[3])
                nc.scalar.dma_start(sup_sb[:, 6, :, :], sup4[6])
                # gpsimd queue
                nc.gpsimd.dma_start(ident[:], ident_d[:])
                nc.gpsimd.dma_start(sel[:], sel_d[:])
                nc.gpsimd.dma_start(sup_sb[:, 2, 0:20, :], sup4[2, :, 0:20, :])
                nc.gpsimd.dma_start(sup_sb[:, 2, 20:, :], sup4[2, :, 20:, :])
                nc.gpsimd.dma_start(qsb[:, 4:BLOC, :], q_d[:, 4:BLOC, :])
                nc.gpsimd.dma_start(sup_sb[:, 4, :, :], sup4[4])
                nc.gpsimd.dma_start(sup_sb[:, 9, :, :], sup4[9])
                # sync queue
                nc.sync.dma_start(wp[:], w_d.rearrange("(ci p) -> p ci", p=128))
                nc.sync.dma_start(sup_sb[:, 1, 0:20, :], sup4[1, :, 0:20, :])
                nc.sync.dma_start(sup_sb[:, 1, 20:, :], sup4[1, :, 20:, :])
                nc.sync.dma_start(sup_sb[:, 5, :, :], sup4[5])
                nc.sync.dma_start(sup_sb[:, 7, :, :], sup4[7])
                nc.sync.dma_start(sup_sb[:, 8, :, :], sup4[8])


            nc.vector.tensor_scalar_mul(wps[:], wp[:], 1.0 / DENOM)
            nc.gpsimd.memset(rall_pk[:], 0.0)

            # ---------------- PE warm-up ----------------
            # ~7us of dummy matmuls bridges the gap until the first support
            # chunks land, releasing the HAM clock gate (cold PE = 1.2 GHz).
            warm = fr_ps.tile([128, 256], f32, tag="score")
            last_warm = None
            for wi in range(10):
                last_warm = nc.tensor.matmul(
                    warm[:], lhsT=ident[:], rhs=warm_src[:],
                    start=(wi == 0), stop=(wi == 9))

            # ---------------- stage S: full support Grams (per way) --------
            def gram_copy(j, gp):
                nc.vector.tensor_copy(
                    rall_pk[:, :, :, j],
                    gp[:, 0:C].rearrange("c (p g) -> c p g", g=4))
                nc.vector.tensor_copy(mcol[:, j:j + 1], gp[:, C:C + 1])

            def gram_part(j, gp, k0, k1, first=False):
                # one accumulation group spans both halves of a split way;
                # skip_group_check lets unrelated PE work (transposes, W)
                # interleave between the halves
                for k in range(k0, k1):
                    g_ = nc.tensor.matmul(
                        gp[:], lhsT=sup_sb[:, j, k, 0:C],
                        rhs=sup_sb[:, j, k, :],
                        start=(k == 0), stop=(k == NCHUNK - 1),
                        skip_group_check=(k != 0 and k != NCHUNK - 1))
                    if first and k == 0:
                        tile.add_dep_helper(
                            g_.ins, last_warm.ins,
                            reason="PE warm-up before stage S")

            def gram(j, first=False):
                gp = gram_ps.tile([C, C + 1], f32, tag="gram")
                gram_part(j, gp, 0, NCHUNK, first=first)
                gram_copy(j, gp)

            # ---------------- stage Q pieces ----------------
            def squares(b):
                sq = scratch.tile([C, HW], bf16, tag="sq")
                nc.scalar.activation(sq[:], qsb[:, b, :], AF.Square,
                                     accum_out=nsq[:, b:b + 1])

            def newton(h):
                # rinv = nsq^(-1/2) by Newton from constant seed (nsq ~ 1024)
                s = slice(2 * h, 2 * h + 2)
                r0 = 2.0 ** -5
                nc.vector.tensor_scalar(tnw[:, s], nsq[:, s],
                                        r0 * r0 * -0.5, 1.5,
                                        ALU.mult, ALU.add)
                nc.vector.tensor_scalar_mul(rin[:, s], tnw[:, s], r0)
                for _ in range(2):
                    nc.vector.tensor_mul(tnw[:, s], rin[:, s], rin[:, s])
                    nc.vector.tensor_mul(tnw[:, s], tnw[:, s], nsq[:, s])
                    nc.vector.tensor_scalar(tnw[:, s], tnw[:, s], -0.5, 1.5,
                                            ALU.mult, ALU.add)
                    nc.vector.tensor_mul(rin[:, s], rin[:, s], tnw[:, s])

            def qnorm(b):
                nc.vector.tensor_scalar_mul(qbf[:, b, :], qsb[:, b, :],
                                            rin[:, b:b + 1])

            def tw(b):
                # transpose qn chunks -> qT (ACT group copies from PSUM),
                # then wqT = qT * w' as ONE broadcast multiply on DVE
                # (per-chunk scalar ops cost ~0.4us fixed each — 20us+
                # across the kernel; the broadcast form is one op per query)
                for g in range(2):
                    pt = tp_ps.tile([128, 4, 128], bf16, tag="tp")
                    for i in range(4):
                        ci = 4 * g + i
                        nc.tensor.transpose(
                            pt[:, i, :],
                            qbf[:, b, 128 * ci:128 * (ci + 1)], ident[:])
                    nc.scalar.activation(qT[:, b, 4 * g:4 * g + 4, :], pt[:],
                                         AF.Copy)
                nc.vector.tensor_tensor(
                    wqT[:, b], qT[:, b],
                    wps[:, :, None].to_broadcast((128, QCH, C)),
                    ALU.mult)

            def wmat(b):
                wpt = w_ps.tile([C, C], f32, tag="wacc")
                for ci in range(QCH):
                    nc.tensor.matmul(wpt[:], lhsT=wqT[:, b, ci, :],
                                     rhs=qT[:, b, ci, :],
                                     start=(ci == 0), stop=(ci == QCH - 1))
                nc.vector.tensor_copy(
                    wsb_pk[:, :, :, b],
                    wpt[:].rearrange("c (p g) -> c p g", g=4))

            # PE stream: Grams in natural way order (arrival ~2.3us/way),
            # query norm chain + transposes/W interleaved as inputs land
            gram(0, first=True)

            def qchain(b0):
                squares(b0)
                squares(b0 + 1)
                newton(b0 // 2)
                qnorm(b0)
                qnorm(b0 + 1)

            qchain(0)
            qchain(2)
            qchain(4)
            qchain(6)

            # Grams in arrival order; tw(b) transposes decoupled from
            # wmat(b) by at least one Gram so the ACT qT copy + DVE wqT
            # multiply complete off the PE critical path
            gram(2)
            gram(1)
            tw(0)
            tw(1)
            wmat(0)
            gram(4)
            wmat(1)
            tw(2)
            tw(3)
            wmat(2)
            gram(3)
            wmat(3)
            tw(4)
            gram(6)
            wmat(4)
            tw(5)
            gram(5)
            wmat(5)
            tw(6)
            gram(7)
            wmat(6)
            tw(7)
            wmat(7)
            gram(8)
            gram(9)

            # ---------------- Frobenius: score[j,b] = <R_j, W_b> -----------
            # 4 c0-columns packed per matmul (d = 4p+g): lhsT/rhs are the
            # contiguous packed tiles; only the 4 diagonal [10,8] blocks of
            # each [128,32] product are wanted (pads are zeroed), folded by
            # the SEL matmuls below.  32 matmuls instead of a 128-long
            # NX-issue-bound c0 loop.  Runs right after the last Gram; the
            # mean-correction chain overlaps on ACT/DVE.
            score4 = fr_ps.tile([64, 32], f32, tag="score")
            for p in range(C // 4):
                nc.tensor.matmul(
                    score4[:],
                    lhsT=rall_pk[:, p, :, :].rearrange("c g j -> c (g j)"),
                    rhs=wsb_pk[:, p, :, :].rearrange("c g b -> c (g b)"),
                    start=(p == 0), stop=(p == C // 4 - 1))
            scr_sb = constp.tile([64, 32], f32, tag="scr_sb")
            nc.vector.tensor_copy(scr_sb[:], score4[:])

            # ---------------- correction: -(1/N) m^T W_b m ----------------
            # mallN = -m/N  (m = per-way row sums) ; msT = m^T
            nc.scalar.activation(mallN[:], mcol[:], AF.Copy,
                                 scale=-1.0 / NTOT)
            mt = tp_ps.tile([WAYS, C], bf16, tag="tp")
            nc.tensor.transpose(mt[:], mcol[:], ident[:])
            nc.vector.tensor_copy(msT[:], mt[:])
            # u[j,(b,d)] = sum_c (-m[j,c]/N) W[b,c,d] ; y = sum_d u * m[j,d]
            for h in range(2):
                up = w_ps.tile([WAYS, BLOC * C // 2], f32, tag="wacc")
                nc.tensor.matmul(
                    up[:], lhsT=mallN[:],
                    rhs=wsb_pk[:, 16 * h:16 * (h + 1), :, :].rearrange(
                        "c p g b -> c (p g b)"),
                    start=True, stop=True)
                nc.vector.tensor_tensor(
                    ytmp[:, :, 64 * h:64 * (h + 1)].rearrange(
                        "j b (p g) -> j p g b", g=4),
                    up[:].rearrange("j (p g b) -> j p g b", g=4, b=BLOC),
                    msT[:, 64 * h:64 * (h + 1)].rearrange(
                        "j (p g) -> j p g", g=4)[:, :, :, None].to_broadcast(
                        (WAYS, 16, 4, BLOC)),
                    ALU.mult)
            nc.vector.tensor_reduce(ysb[:], ytmp[:],
                                    axis=mybir.AxisListType.X,
                                    op=ALU.add)

            # fold the 4 diagonal blocks on the PE, then add the correction
            fin_ps = w_ps.tile([WAYS, BLOC], f32, tag="wacc")
            for g in range(4):
                nc.tensor.matmul(fin_ps[:], lhsT=sel[:, g, :],
                                 rhs=scr_sb[:, 8 * g:8 * g + 8],
                                 start=(g == 0), stop=(g == 3))
            nc.vector.tensor_add(fin[:], fin_ps[:], ysb[:])
            nc.sync.dma_start(out_d[:], fin[:])

    nc.compile()
    return nc


def _get_program():
    if "nc" not in _CACHE:
        _CACHE["nc"] = _build_program()
    return _CACHE["nc"]


def _make_in_maps(q, support, conv_w):
    import ml_dtypes
    q = np.asarray(q, dtype=np.float32).reshape(B, C, HW)
    qb = q.astype(ml_dtypes.bfloat16)
    # sample-major support: [ways, sample, C] with sample = (shot, pixel),
    # chunked as sample = 128*k + p, laid out [ways, p, k, c] with a ones
    # column at c=C (feeds the row-sum side of the Gram matmul)
    s = np.asarray(support, dtype=np.float32).reshape(WAYS, SHOTS, C, HW)
    s = s.transpose(0, 1, 3, 2).reshape(WAYS, NTOT, C)
    s = s.reshape(WAYS, NCHUNK, 128, C).transpose(0, 2, 1, 3)
    sp = np.empty((WAYS, 128, NCHUNK, C + 1), dtype=ml_dtypes.float8_e4m3)
    sp[..., :C] = s.astype(ml_dtypes.float8_e4m3)
    sp[..., C] = 1.0
    sp = np.ascontiguousarray(sp.reshape(WAYS, 128, NCHUNK * (C + 1)))
    w = np.ascontiguousarray(np.asarray(conv_w, dtype=np.float32))
    in_maps = []
    for k in range(NCORES):
        in_maps.append({
            "q": np.ascontiguousarray(
                qb[k * BLOC:(k + 1) * BLOC].transpose(1, 0, 2)),
            "support": sp,
            "conv_w": w,
        })
    return in_maps


def _run(in_maps, trace=False):
    from concourse.bass_utils import run_bass_kernel_spmd
    nc = _get_program()
    return run_bass_kernel_spmd(nc, in_maps, list(range(NCORES)), trace=trace)


def kernel(q, support, conv_w):
    res = _run(_make_in_maps(q, support, conv_w))
    out = np.concatenate(
        [res.results[k]["out"].T for k in range(NCORES)], axis=0)
    return np.ascontiguousarray(out.astype(np.float32))
